# revision 9
# baseline (speedup 1.0000x reference)
"""Tversky-style mismatch loss on Trainium2 (Bass/Tile), 8-core data-parallel.

Full inputs: net_out/target/max_positiones, each [8, 16, 512, 512] f32.
Sharding: batch dim B=8 across 8 NeuronCores (1 image per core).

Per (image, class) plane we need only four reductions:
    tn = sum(target * net_out)        (via DVE tensor_tensor_reduce)
    t  = sum(target)                  (via PE matmul against ones)
    n  = sum(net_out)                 (via PE matmul against ones)
    m  = sum(max_positiones)          (via PE matmul against ones)
since fn = t - tn, fp = n - tn, and active = (t > 0) | (m > 0) (masks are
0/1-valued so sum>0 <=> max>0).  The tiny [8,16] -> scalar tail runs on host.
"""

import sys

import numpy as np

if "/opt/trn_rl_repo" not in sys.path:
    sys.path.insert(0, "/opt/trn_rl_repo")

B, C, H, W = 8, 16, 512, 512
NCORES = 8
P = 128
FREE = H * W // P  # 2048 f32 per partition per plane
CHUNK = 512  # max fp32 moving free dim per matmul
NCHUNK = FREE // CHUNK  # 4

_CACHE = {}


def _build(C=C, H=H, W=W, debug=False, num_devices=NCORES):
    import concourse.bacc as bacc
    import concourse.mybir as mybir
    import concourse.tile as tile

    P = 128
    FREE = H * W // P
    CHUNK = min(512, FREE)
    NCHUNK = FREE // CHUNK

    f32 = mybir.dt.float32
    bf16 = mybir.dt.bfloat16
    nc = bacc.Bacc(
        "TRN2", target_bir_lowering=False, debug=debug, num_devices=num_devices
    )

    t_in = nc.dram_tensor("t_in", [C, H, W], f32, kind="ExternalInput")
    n_in = nc.dram_tensor("n_in", [C, H, W], f32, kind="ExternalInput")
    m_in = nc.dram_tensor("m_in", [C, H, W], f32, kind="ExternalInput")
    out_tn = nc.dram_tensor("out_tn", [1, C], f32, kind="ExternalOutput")
    out_tnm = nc.dram_tensor("out_tnm", [C, 3], f32, kind="ExternalOutput")

    # pair of planes g as [128 partitions, 2 x 2048 contiguous f32]
    CPT = 2 if C % 2 == 0 else 1  # planes per DMA tile
    NT = C // CPT
    t_r = t_in.ap().rearrange("(g c) (p a) w -> g p c (a w)", c=CPT, p=P)
    n_r = n_in.ap().rearrange("(g c) (p a) w -> g p c (a w)", c=CPT, p=P)
    m_r = m_in.ap().rearrange("(g c) (p a) w -> g p c (a w)", c=CPT, p=P)

    with tile.TileContext(nc) as tc:
        with (
            tc.tile_pool(name="consts", bufs=1) as consts,
            tc.tile_pool(name="tp", bufs=3) as tp,
            tc.tile_pool(name="npool", bufs=3) as npool,
            tc.tile_pool(name="mp", bufs=3) as mp,
            tc.tile_pool(name="sp", bufs=2) as sp,
            tc.tile_pool(name="outp", bufs=1) as outp,
            tc.tile_pool(name="psum", bufs=1, space="PSUM") as psum,
        ):
            ones = consts.tile([P, 1], f32)
            nc.vector.memset(ones[:], 1.0)
            # G[:, C-1] = 1, rest 0.  lhsT window G[:, C-1-c : 2C-1-c] is a
            # [P, C] matrix whose column c is all-ones -> plane c's column
            # sums land in PSUM partition row c, other rows accumulate +0.
            G = consts.tile([P, 2 * C - 1], bf16)
            nc.vector.memset(G[:], 0.0)
            nc.vector.memset(G[:, C - 1 : C], 1.0)
            acc = consts.tile([P, C], f32)  # per-plane partition-partials of tn

            ps_t = psum.tile([C, CHUNK], f32)
            ps_n = psum.tile([C, CHUNK], f32)
            ps_m = psum.tile([C, CHUNK], f32)
            ps_tn = psum.tile([1, C], f32)

            for g in range(NT):
                # SWDGE DMAs cast f32 -> bf16 in flight (HWDGE can't cast).
                # target/max_positiones are 0/1-valued so bf16 is exact;
                # net_out's per-plane sums only pick up ~1e-6 rel error.
                tt = tp.tile([P, CPT * FREE], bf16)
                nc.gpsimd.dma_start(
                    tt[:].rearrange("p (c f) -> p c f", c=CPT), t_r[g]
                )
                nt = npool.tile([P, CPT * FREE], bf16)
                nc.gpsimd.dma_start(
                    nt[:].rearrange("p (c f) -> p c f", c=CPT), n_r[g]
                )
                mt = mp.tile([P, CPT * FREE], bf16)
                nc.gpsimd.dma_start(
                    mt[:].rearrange("p (c f) -> p c f", c=CPT), m_r[g]
                )

                for cc in range(CPT):
                    c = g * CPT + cc
                    fsl = slice(cc * FREE, (cc + 1) * FREE)
                    sc = sp.tile([P, FREE], bf16)
                    # out = (t * 1.0) * n; accum_out = per-partition row sum.
                    nc.vector.scalar_tensor_tensor(
                        out=sc[:],
                        in0=tt[:, fsl],
                        scalar=1.0,
                        in1=nt[:, fsl],
                        op0=mybir.AluOpType.mult,
                        op1=mybir.AluOpType.mult,
                        accum_out=acc[:, c : c + 1],
                    )

                    w = G[:, C - 1 - c : 2 * C - 1 - c]
                    for k in range(NCHUNK):
                        first = c == 0 and k == 0
                        last = c == C - 1 and k == NCHUNK - 1
                        sl = slice(cc * FREE + k * CHUNK, cc * FREE + (k + 1) * CHUNK)
                        nc.tensor.matmul(
                            ps_t[:, :], w, tt[:, sl], start=first, stop=last
                        )
                        nc.tensor.matmul(
                            ps_n[:, :], w, nt[:, sl], start=first, stop=last
                        )
                        nc.tensor.matmul(
                            ps_m[:, :], w, mt[:, sl], start=first, stop=last
                        )

            # partition-axis total of the tn partials: [128, C] -> [1, C]
            nc.tensor.matmul(ps_tn[:, :], ones[:], acc[:], start=True, stop=True)

            sb_tnm = outp.tile([C, 3], f32)
            nc.vector.reduce_sum(sb_tnm[:, 0:1], ps_t[:], axis=mybir.AxisListType.X)
            nc.vector.reduce_sum(sb_tnm[:, 1:2], ps_n[:], axis=mybir.AxisListType.X)
            nc.vector.reduce_sum(sb_tnm[:, 2:3], ps_m[:], axis=mybir.AxisListType.X)
            sb_tn = outp.tile([1, C], f32)
            nc.vector.tensor_copy(sb_tn[:], ps_tn[:])

            nc.sync.dma_start(out_tnm.ap(), sb_tnm[:])
            nc.sync.dma_start(out_tn.ap(), sb_tn[:])

    nc.compile()
    return nc


def _get_nc():
    if "nc" not in _CACHE:
        _CACHE["nc"] = _build()
    return _CACHE["nc"]


def _run(net_out, target, max_positiones, trace=False):
    from concourse.bass_utils import run_bass_kernel_spmd

    nc = _get_nc()
    in_maps = []
    for i in range(NCORES):
        in_maps.append(
            {
                "t_in": np.ascontiguousarray(target[i]),
                "n_in": np.ascontiguousarray(net_out[i]),
                "m_in": np.ascontiguousarray(max_positiones[i]),
            }
        )
    res = run_bass_kernel_spmd(
        nc, in_maps, core_ids=list(range(NCORES)), trace=trace
    )
    return res


def _finish(results):
    # results: list (per core) of {"out_tn": [1,C], "out_tnm": [C,3]}
    tn = np.stack([r["out_tn"][0] for r in results]).astype(np.float64)  # [B,C]
    tnm = np.stack([r["out_tnm"] for r in results]).astype(np.float64)  # [B,C,3]
    st, sn, sm = tnm[..., 0], tnm[..., 1], tnm[..., 2]

    b2 = 1.5 * 1.5
    w1 = b2 / (1.0 + b2)
    w2 = 1.0 / (1.0 + b2)
    molecule = tn
    fn = st - tn
    fp = sn - tn
    loss = 1.0 - molecule / (molecule + w1 * fn + w2 * fp)
    active = (st > 0) | (sm > 0)
    losses = np.where(active, loss, 0.0)
    cnt = np.sum(losses != 0, axis=1).astype(np.float64)
    img_losses = np.sum(losses, axis=1) / cnt
    out = np.sum(img_losses) / img_losses.shape[0]
    return np.float32(out)


def kernel(net_out, target, max_positiones):
    net_out = np.asarray(net_out, dtype=np.float32)
    target = np.asarray(target, dtype=np.float32)
    max_positiones = np.asarray(max_positiones, dtype=np.float32)
    res = _run(net_out, target, max_positiones, trace=False)
    return _finish(res.results)
